# revision 8
# baseline (speedup 1.0000x reference)
"""Trainium2 Bass kernel for DGL HGNNConv-style hypergraph message passing.

Computation (see problem reference):
    Xp = X @ Wlin                                   # [N, 128] @ [128, 128]
    Xe = segment_sum(Xp[g1_src], g1_dst, 25000)     # node -> hyperedge
    Xe = Xe * degE * W
    Xv = segment_sum(Xe[g1_dst], g1_src, 100000)    # hyperedge -> node
    Xv = Xv * degV

Distribution strategy (8 NeuronCores, node-range sharding):
  - Core m owns node rows [m*12500, (m+1)*12500) and all nnz entries whose
    src falls in that range (both stages use the same entry sharding).
  - Projection: each core computes Xp (bf16) for its own node shard only.
  - Stage 1: per-core entries sorted by dst; rows of the local Xp gathered
    per entry (dma_gather), segment-summed into a full-range partial Xe
    via data-dependent one-hot matmuls (PSUM accumulation), scaled by
    degE*W, then AllReduced across cores (4 chunks, overlapped).
  - Stage 2: per-core entries sorted by src; rows of the reduced Xe
    gathered per entry, segment-summed into the core's node tile,
    scaled by degV, written to the core's output shard.

Performance notes:
  - dma_gather descriptor generation runs on one gpsimd (Q7) core-pair
    selected by queue_num; round-robining calls across the 4 SWDGE queues
    runs four generations concurrently (~3.6x gather throughput).
  - Gather tables (Xp, reduced Xe) are stored bf16: halves gather HBM
    traffic and feeds the one-hot matmuls bf16 inputs directly (1-pass PE
    matmuls instead of 4-pass fp32).
  - The AllReduce writes straight into the shared full-Xe tensor slice
    per chunk (no DRAM->DRAM copy).

Segment-sum-as-matmul: for each chunk of 128 gathered rows G [128e x 128f]
and one-hot S [128e x 128s] (S[k, m] = 1 iff entry k belongs to local
segment m, built on-chip with is_equal against an iota tile), the matmul
S^T @ G accumulates the chunk into the 128-segment PSUM tile.
"""

import ml_dtypes
import numpy as np

import concourse.bass as bass
import concourse.bacc as bacc
import concourse.tile as tile
import concourse.mybir as mybir
from concourse.bass_utils import run_bass_kernel_spmd
from concourse.masks import make_identity

P = 128
N_CORES = 8
N_QUEUES = 4

N_NODES = 100000
N_HEDGES = 25000
IN_CH = 128
OUT_CH = 128
N_AR_CHUNKS = 4  # AllReduce split for overlap with stage-1 compute
USE_COLLECTIVE = True

BF16 = ml_dtypes.bfloat16


def _cdiv(a, b):
    return (a + b - 1) // b


def _wrap_idx16(idx_flat: np.ndarray) -> np.ndarray:
    """Pack a flat index array into the [128, n/16] int16 SBUF layout used
    by dma_gather: flat index i -> partition i%16, column i//16, replicated
    across the eight 16-partition stripes."""
    n = idx_flat.shape[0]
    assert n % 16 == 0
    blk = idx_flat.astype(np.int16).reshape(n // 16, 16).T  # [16, cols]
    return np.tile(blk, (8, 1))  # [128, cols]


def _prep_stage(tile_key, gather_idx, local_id, n_tiles, n_cores):
    """Build per-core padded gather-index / segment-id arrays with a chunk
    schedule that is uniform across cores (SPMD requires one program).

    tile_key: per-core arrays with the tile id per entry (nondecreasing).
    Returns (chunks [n_tiles], idx_wrapped list, ids list).
    """
    counts = np.zeros((n_cores, n_tiles), dtype=np.int64)
    slices = []
    for c in range(n_cores):
        bounds = np.searchsorted(tile_key[c], np.arange(n_tiles + 1),
                                 side="left")
        counts[c] = bounds[1:] - bounds[:-1]
        slices.append(bounds)
    chunks = np.maximum(1, _cdiv(counts.max(axis=0), P)).astype(np.int64)
    total_chunks = int(chunks.sum())
    total = total_chunks * P
    co = np.concatenate([[0], np.cumsum(chunks)])

    idx_w, ids_w = [], []
    for c in range(n_cores):
        idx_flat = np.zeros(total, dtype=np.int16)
        ids_flat = np.full(total, -1.0, dtype=np.float32)
        bounds = slices[c]
        gi, li = gather_idx[c], local_id[c]
        for t in range(n_tiles):
            lo, hi = bounds[t], bounds[t + 1]
            cnt = hi - lo
            base = int(co[t]) * P
            idx_flat[base:base + cnt] = gi[lo:hi]
            ids_flat[base:base + cnt] = li[lo:hi]
        idx_w.append(_wrap_idx16(idx_flat))
        ids_w.append(np.ascontiguousarray(
            ids_flat.reshape(total_chunks, P).T).astype(BF16))
    return chunks, idx_w, ids_w


def _build_program(ns_pad, seg_pad, chunks1, chunks2, n_cores):
    """Emit the SPMD Bass program (identical for all cores)."""
    n_tiles_proj = ns_pad // P
    n_seg_tiles = seg_pad // P
    n_node_tiles = ns_pad // P
    tc1 = int(chunks1.sum())
    tc2 = int(chunks2.sum())
    co1 = np.concatenate([[0], np.cumsum(chunks1)]).astype(int)
    co2 = np.concatenate([[0], np.cumsum(chunks2)]).astype(int)

    nc = bacc.Bacc("TRN2", target_bir_lowering=False, debug=False,
                   num_devices=n_cores, num_swdge_queues=N_QUEUES)

    x_shard = nc.dram_tensor("x_shard", [ns_pad, IN_CH], mybir.dt.float32,
                             kind="ExternalInput")
    wlin = nc.dram_tensor("wlin", [IN_CH, OUT_CH], mybir.dt.float32,
                          kind="ExternalInput")
    dege_r = nc.dram_tensor("dege_r", [P, n_seg_tiles], mybir.dt.float32,
                            kind="ExternalInput")
    degv_r = nc.dram_tensor("degv_r", [P, n_node_tiles], mybir.dt.float32,
                            kind="ExternalInput")
    colidx_in = nc.dram_tensor("colidx", [P, P], mybir.dt.bfloat16,
                               kind="ExternalInput")
    idx1_in = nc.dram_tensor("idx1", [P, tc1 * 8], mybir.dt.int16,
                             kind="ExternalInput")
    ids1_in = nc.dram_tensor("ids1", [P, tc1], mybir.dt.bfloat16,
                             kind="ExternalInput")
    idx2_in = nc.dram_tensor("idx2", [P, tc2 * 8], mybir.dt.int16,
                             kind="ExternalInput")
    ids2_in = nc.dram_tensor("ids2", [P, tc2], mybir.dt.bfloat16,
                             kind="ExternalInput")
    out_shard = nc.dram_tensor("out_shard", [ns_pad, OUT_CH],
                               mybir.dt.float32, kind="ExternalOutput")

    # AllReduce chunk row ranges (in seg tiles)
    n_ar = min(N_AR_CHUNKS, n_seg_tiles)
    q_tiles = [n_seg_tiles // n_ar] * n_ar
    for i in range(n_seg_tiles % n_ar):
        q_tiles[i] += 1
    q_tile_lo = np.concatenate([[0], np.cumsum(q_tiles)]).astype(int)

    qctr = [0]  # SWDGE queue rotation

    with tile.TileContext(nc) as tc:
        with (
            tc.tile_pool(name="const", bufs=1) as cpool,
            tc.tile_pool(name="work", bufs=6) as work,
            tc.tile_pool(name="small", bufs=2) as small,
            tc.tile_pool(name="psum", bufs=2, space="PSUM") as psum,
            tc.tile_pool(name="psacc", bufs=4, space="PSUM") as psacc,
            tc.tile_pool(name="dram", bufs=1, space="DRAM") as dram,
        ):
            # ---- preloads ----
            idx1_sb = cpool.tile([P, tc1 * 8], mybir.dt.int16)
            nc.sync.dma_start(idx1_sb[:], idx1_in[:])
            ids1_sb = cpool.tile([P, tc1], mybir.dt.bfloat16)
            nc.sync.dma_start(ids1_sb[:], ids1_in[:])
            idx2_sb = cpool.tile([P, tc2 * 8], mybir.dt.int16)
            nc.sync.dma_start(idx2_sb[:], idx2_in[:])
            ids2_sb = cpool.tile([P, tc2], mybir.dt.bfloat16)
            nc.sync.dma_start(ids2_sb[:], ids2_in[:])
            colidx_sb = cpool.tile([P, P], mybir.dt.bfloat16)
            nc.sync.dma_start(colidx_sb[:], colidx_in[:])
            wlin_f32 = cpool.tile([P, OUT_CH], mybir.dt.float32)
            nc.sync.dma_start(wlin_f32[:], wlin[:])
            wlin_sb = cpool.tile([P, OUT_CH], mybir.dt.bfloat16)
            nc.vector.tensor_copy(wlin_sb[:], wlin_f32[:])
            degv_sb = cpool.tile([P, n_node_tiles], mybir.dt.float32)
            nc.sync.dma_start(degv_sb[:], degv_r[:])
            scale_e = cpool.tile([P, n_seg_tiles], mybir.dt.float32)
            nc.sync.dma_start(scale_e[:], dege_r[:])
            ident = cpool.tile([P, P], mybir.dt.float32)
            make_identity(nc, ident[:])
            colidx3 = colidx_sb[:].rearrange("p (o e) -> p o e", o=1)

            xp_local = dram.tile([ns_pad, OUT_CH], mybir.dt.bfloat16)
            xe_part = [
                dram.tile([q_tiles[q] * P, OUT_CH], mybir.dt.bfloat16,
                          name=f"xe_part{q}")
                for q in range(n_ar)
            ]
            xe_full = dram.tile([seg_pad, OUT_CH], mybir.dt.bfloat16)
            xe_red = [
                dram.tile([q_tiles[q] * P, OUT_CH], mybir.dt.bfloat16,
                          name=f"xe_red{q}", addr_space="Shared")
                for q in range(n_ar)
            ]

            # ---- projection: xp_local = bf16(x_shard @ wlin) ----
            for t in range(n_tiles_proj):
                rows = slice(t * P, (t + 1) * P)
                xt = small.tile([P, IN_CH], mybir.dt.float32, tag="xt")
                nc.sync.dma_start(xt[:], x_shard[rows, :])
                tp = psum.tile([P, P], mybir.dt.float32, space="PSUM",
                               tag="tp")
                nc.tensor.transpose(tp[:], xt[:], ident[:])
                xts = small.tile([P, P], mybir.dt.bfloat16, tag="xts")
                nc.vector.tensor_copy(xts[:], tp[:])
                xpp = psum.tile([P, OUT_CH], mybir.dt.float32, space="PSUM",
                                tag="xpp")
                nc.tensor.matmul(xpp[:], xts[:], wlin_sb[:], start=True,
                                 stop=True)
                xps = small.tile([P, OUT_CH], mybir.dt.bfloat16, tag="xps")
                nc.vector.tensor_copy(xps[:], xpp[:])
                nc.sync.dma_start(xp_local[rows, :], xps[:])

            # ---- generic segment-sum stage ----
            def seg_stage(t, ch, co_t, idx_sb, ids_sb, src_ap, scale_ap,
                          out_ap, out_dtype):
                n = ch * P
                g = work.tile([P, n], mybir.dt.bfloat16, tag="g")
                # split gathers into balanced calls of <=8 chunks (1024
                # descriptors: 64 ring entries/engine, under the 128-entry
                # SWDGE ring), round-robined over the 4 SWDGE queues so
                # descriptor generation runs on all gpsimd core-pairs
                # concurrently
                n_calls = _cdiv(ch, 8)
                base_w, rem_w = divmod(ch, n_calls)
                c0 = 0
                for ci in range(n_calls):
                    cw = base_w + (1 if ci < rem_w else 0)
                    gs = g[:, c0 * P:(c0 + cw) * P].rearrange(
                        "p (c e) -> p c e", e=P)
                    nc.gpsimd.dma_gather(
                        gs, src_ap,
                        idx_sb[:, (co_t + c0) * 8:(co_t + c0 + cw) * 8],
                        cw * P, cw * P, P,
                        queue_num=qctr[0] % N_QUEUES)
                    qctr[0] += 1
                    c0 += cw
                s = work.tile([P, n], mybir.dt.bfloat16, tag="s")
                s3 = s[:].rearrange("p (c e) -> p c e", e=P)
                nc.vector.tensor_tensor(
                    out=s3,
                    in0=ids_sb[:, co_t:co_t + ch].to_broadcast([P, ch, P]),
                    in1=colidx3.to_broadcast([P, ch, P]),
                    op=mybir.AluOpType.is_equal,
                )
                acc = psacc.tile([P, OUT_CH], mybir.dt.float32,
                                 space="PSUM", tag="acc")
                for c in range(ch):
                    nc.tensor.matmul(
                        acc[:], s[:, c * P:(c + 1) * P],
                        g[:, c * P:(c + 1) * P],
                        start=(c == 0), stop=(c == ch - 1))
                ev = work.tile([P, OUT_CH], out_dtype, tag="ev")
                nc.vector.tensor_scalar_mul(ev[:], acc[:], scale_ap)
                nc.sync.dma_start(out_ap, ev[:])

            # ---- stage 1 (+ chunked AllReduce) ----
            for q in range(n_ar):
                for t in range(q_tile_lo[q], q_tile_lo[q + 1]):
                    trel = t - q_tile_lo[q]
                    seg_stage(
                        t, int(chunks1[t]), int(co1[t]), idx1_sb, ids1_sb,
                        xp_local[:], scale_e[:, t:t + 1],
                        xe_part[q][trel * P:(trel + 1) * P, :],
                        mybir.dt.bfloat16)
                if USE_COLLECTIVE:
                    nc.gpsimd.collective_compute(
                        "AllReduce", mybir.AluOpType.add,
                        replica_groups=[list(range(n_cores))],
                        ins=[xe_part[q].opt()],
                        outs=[xe_red[q].opt()],
                    )
                    nc.sync.dma_start(
                        xe_full[q_tile_lo[q] * P:q_tile_lo[q + 1] * P, :],
                        xe_red[q][:])
                else:
                    nc.sync.dma_start(
                        xe_full[q_tile_lo[q] * P:q_tile_lo[q + 1] * P, :],
                        xe_part[q][:])

            # ---- stage 2 ----
            for t in range(n_node_tiles):
                seg_stage(
                    t, int(chunks2[t]), int(co2[t]), idx2_sb, ids2_sb,
                    xe_full[:], degv_sb[:, t:t + 1],
                    out_shard[t * P:(t + 1) * P, :], mybir.dt.float32)

    nc.compile()
    return nc


def _host_prep(X, Wlin, degE, degV, W, g1_src, g1_dst, n_cores=N_CORES):
    ns = N_NODES // n_cores
    ns_pad = _cdiv(ns, P) * P
    n_seg_tiles = _cdiv(N_HEDGES, P)
    seg_pad = n_seg_tiles * P
    n_node_tiles = ns_pad // P

    core_of = g1_src // ns

    # stage 1: per core, sorted by dst
    o1 = np.lexsort((g1_dst, core_of))
    src1, dst1, c1 = g1_src[o1], g1_dst[o1], core_of[o1]
    cb1 = np.searchsorted(c1, np.arange(n_cores + 1))
    tile_key1, gidx1, lid1 = [], [], []
    for c in range(n_cores):
        lo, hi = cb1[c], cb1[c + 1]
        d = dst1[lo:hi]
        tile_key1.append(d // P)
        gidx1.append(src1[lo:hi] - c * ns)
        lid1.append((d % P).astype(np.float32))
    chunks1, idx1_w, ids1_w = _prep_stage(
        tile_key1, gidx1, lid1, n_seg_tiles, n_cores)

    # stage 2: per core, sorted by src
    o2 = np.argsort(g1_src, kind="stable")
    src2, dst2 = g1_src[o2], g1_dst[o2]
    cb2 = np.searchsorted(src2, np.arange(n_cores + 1) * ns)
    tile_key2, gidx2, lid2 = [], [], []
    for c in range(n_cores):
        lo, hi = cb2[c], cb2[c + 1]
        s_local = src2[lo:hi] - c * ns
        tile_key2.append(s_local // P)
        gidx2.append(dst2[lo:hi])
        lid2.append((s_local % P).astype(np.float32))
    chunks2, idx2_w, ids2_w = _prep_stage(
        tile_key2, gidx2, lid2, n_node_tiles, n_cores)

    # rearranged scale vectors: column t holds values for tile t's rows;
    # degE is pre-multiplied by W (elementwise hyperedge weight)
    def col_tiles(v, pad_rows):
        vp = np.zeros(pad_rows, dtype=np.float32)
        vp[:v.shape[0]] = v.reshape(-1)
        return np.ascontiguousarray(vp.reshape(pad_rows // P, P).T)

    dege_r = col_tiles((degE * W).astype(np.float32), seg_pad)
    colidx = np.broadcast_to(
        np.arange(P, dtype=np.float32), (P, P)).astype(BF16)

    in_maps = []
    for c in range(n_cores):
        xs = np.zeros((ns_pad, IN_CH), dtype=np.float32)
        xs[:ns] = X[c * ns:(c + 1) * ns]
        in_maps.append({
            "x_shard": xs,
            "wlin": np.ascontiguousarray(Wlin, dtype=np.float32),
            "dege_r": dege_r,
            "degv_r": col_tiles(degV[c * ns:(c + 1) * ns], ns_pad),
            "colidx": np.ascontiguousarray(colidx),
            "idx1": idx1_w[c],
            "ids1": ids1_w[c],
            "idx2": idx2_w[c],
            "ids2": ids2_w[c],
        })
    return in_maps, chunks1, chunks2, ns, ns_pad, seg_pad


def run_impl(inputs: dict, trace: bool = False):
    X = np.asarray(inputs["X"], dtype=np.float32)
    Wlin = np.asarray(inputs["Wlin"], dtype=np.float32)
    degE = np.asarray(inputs["degE"], dtype=np.float32)
    degV = np.asarray(inputs["degV"], dtype=np.float32)
    W = np.asarray(inputs["W"], dtype=np.float32)
    g1_src = np.asarray(inputs["g1_src"], dtype=np.int64)
    g1_dst = np.asarray(inputs["g1_dst"], dtype=np.int64)

    in_maps, chunks1, chunks2, ns, ns_pad, seg_pad = _host_prep(
        X, Wlin, degE, degV, W, g1_src, g1_dst)
    nc = _build_program(ns_pad, seg_pad, chunks1, chunks2, N_CORES)
    res = run_bass_kernel_spmd(nc, in_maps, core_ids=list(range(N_CORES)),
                               trace=trace)
    out = np.concatenate(
        [res.results[c]["out_shard"][:ns] for c in range(N_CORES)], axis=0)
    return out, res


def kernel(**inputs) -> np.ndarray:
    out, _ = run_impl(inputs, trace=False)
    return out


# revision 9
# speedup vs baseline: 1.0767x; 1.0767x over previous
"""Trainium2 Bass kernel for DGL HGNNConv-style hypergraph message passing.

Computation (see problem reference):
    Xp = X @ Wlin                                   # [N, 128] @ [128, 128]
    Xe = segment_sum(Xp[g1_src], g1_dst, 25000)     # node -> hyperedge
    Xe = Xe * degE * W
    Xv = segment_sum(Xe[g1_dst], g1_src, 100000)    # hyperedge -> node
    Xv = Xv * degV

Distribution strategy (8 NeuronCores, node-range sharding):
  - Core m owns node rows [m*12500, (m+1)*12500) and all nnz entries whose
    src falls in that range (both stages use the same entry sharding).
  - Projection: each core computes Xp (bf16) for its own node shard only.
  - Stage 1: per-core entries sorted by dst; rows of the local Xp gathered
    per entry (dma_gather), segment-summed into a full-range partial Xe
    via data-dependent one-hot matmuls (PSUM accumulation), scaled by
    degE*W, then AllReduced across cores (4 chunks, overlapped).
  - Stage 2: per-core entries sorted by src; rows of the reduced Xe
    gathered per entry, segment-summed into the core's node tile,
    scaled by degV, written to the core's output shard.

Performance notes:
  - dma_gather descriptor generation runs on one gpsimd (Q7) core-pair
    selected by queue_num; round-robining calls across the 4 SWDGE queues
    runs four generations concurrently (~3.6x gather throughput).
  - Gather tables (Xp, reduced Xe) are stored bf16: halves gather HBM
    traffic and feeds the one-hot matmuls bf16 inputs directly (1-pass PE
    matmuls instead of 4-pass fp32).
  - The AllReduce writes straight into the shared full-Xe tensor slice
    per chunk (no DRAM->DRAM copy).

Segment-sum-as-matmul: for each chunk of 128 gathered rows G [128e x 128f]
and one-hot S [128e x 128s] (S[k, m] = 1 iff entry k belongs to local
segment m, built on-chip with is_equal against an iota tile), the matmul
S^T @ G accumulates the chunk into the 128-segment PSUM tile.
"""

import ml_dtypes
import numpy as np

import concourse.bass as bass
import concourse.bacc as bacc
import concourse.tile as tile
import concourse.mybir as mybir
from concourse.bass_utils import run_bass_kernel_spmd
from concourse.masks import make_identity

P = 128
N_CORES = 8
N_QUEUES = 4

N_NODES = 100000
N_HEDGES = 25000
IN_CH = 128
OUT_CH = 128
N_AR_CHUNKS = 4  # AllReduce split for overlap with stage-1 compute
USE_COLLECTIVE = True

BF16 = ml_dtypes.bfloat16


def _cdiv(a, b):
    return (a + b - 1) // b


def _wrap_idx16(idx_flat: np.ndarray) -> np.ndarray:
    """Pack a flat index array into the [128, n/16] int16 SBUF layout used
    by dma_gather: flat index i -> partition i%16, column i//16, replicated
    across the eight 16-partition stripes."""
    n = idx_flat.shape[0]
    assert n % 16 == 0
    blk = idx_flat.astype(np.int16).reshape(n // 16, 16).T  # [16, cols]
    return np.tile(blk, (8, 1))  # [128, cols]


def _prep_stage(tile_key, gather_idx, local_id, n_tiles, n_cores):
    """Build per-core padded gather-index / segment-id arrays with a chunk
    schedule that is uniform across cores (SPMD requires one program).

    tile_key: per-core arrays with the tile id per entry (nondecreasing).
    Returns (chunks [n_tiles], idx_wrapped list, ids list).
    """
    counts = np.zeros((n_cores, n_tiles), dtype=np.int64)
    slices = []
    for c in range(n_cores):
        bounds = np.searchsorted(tile_key[c], np.arange(n_tiles + 1),
                                 side="left")
        counts[c] = bounds[1:] - bounds[:-1]
        slices.append(bounds)
    chunks = np.maximum(1, _cdiv(counts.max(axis=0), P)).astype(np.int64)
    total_chunks = int(chunks.sum())
    total = total_chunks * P
    co = np.concatenate([[0], np.cumsum(chunks)])

    idx_w, ids_w = [], []
    for c in range(n_cores):
        idx_flat = np.zeros(total, dtype=np.int16)
        ids_flat = np.full(total, -1.0, dtype=np.float32)
        bounds = slices[c]
        gi, li = gather_idx[c], local_id[c]
        for t in range(n_tiles):
            lo, hi = bounds[t], bounds[t + 1]
            cnt = hi - lo
            base = int(co[t]) * P
            idx_flat[base:base + cnt] = gi[lo:hi]
            ids_flat[base:base + cnt] = li[lo:hi]
        idx_w.append(_wrap_idx16(idx_flat))
        ids_w.append(np.ascontiguousarray(
            ids_flat.reshape(total_chunks, P).T).astype(BF16))
    return chunks, idx_w, ids_w


def _build_program(ns_pad, seg_pad, chunks1, chunks2, n_cores):
    """Emit the SPMD Bass program (identical for all cores)."""
    n_tiles_proj = ns_pad // P
    n_seg_tiles = seg_pad // P
    n_node_tiles = ns_pad // P
    tc1 = int(chunks1.sum())
    tc2 = int(chunks2.sum())
    co1 = np.concatenate([[0], np.cumsum(chunks1)]).astype(int)
    co2 = np.concatenate([[0], np.cumsum(chunks2)]).astype(int)

    nc = bacc.Bacc("TRN2", target_bir_lowering=False, debug=False,
                   num_devices=n_cores, num_swdge_queues=N_QUEUES)

    x_shard = nc.dram_tensor("x_shard", [ns_pad, IN_CH], mybir.dt.float32,
                             kind="ExternalInput")
    wlin = nc.dram_tensor("wlin", [IN_CH, OUT_CH], mybir.dt.float32,
                          kind="ExternalInput")
    dege_r = nc.dram_tensor("dege_r", [P, n_seg_tiles], mybir.dt.float32,
                            kind="ExternalInput")
    degv_r = nc.dram_tensor("degv_r", [P, n_node_tiles], mybir.dt.float32,
                            kind="ExternalInput")
    colidx_in = nc.dram_tensor("colidx", [P, P], mybir.dt.bfloat16,
                               kind="ExternalInput")
    idx1_in = nc.dram_tensor("idx1", [P, tc1 * 8], mybir.dt.int16,
                             kind="ExternalInput")
    ids1_in = nc.dram_tensor("ids1", [P, tc1], mybir.dt.bfloat16,
                             kind="ExternalInput")
    idx2_in = nc.dram_tensor("idx2", [P, tc2 * 8], mybir.dt.int16,
                             kind="ExternalInput")
    ids2_in = nc.dram_tensor("ids2", [P, tc2], mybir.dt.bfloat16,
                             kind="ExternalInput")
    out_shard = nc.dram_tensor("out_shard", [ns_pad, OUT_CH],
                               mybir.dt.float32, kind="ExternalOutput")

    # AllReduce chunk row ranges (in seg tiles)
    n_ar = min(N_AR_CHUNKS, n_seg_tiles)
    q_tiles = [n_seg_tiles // n_ar] * n_ar
    for i in range(n_seg_tiles % n_ar):
        q_tiles[i] += 1
    q_tile_lo = np.concatenate([[0], np.cumsum(q_tiles)]).astype(int)

    qctr = [0]  # SWDGE queue rotation

    with tile.TileContext(nc) as tc:
        with (
            tc.tile_pool(name="const", bufs=1) as cpool,
            tc.tile_pool(name="work", bufs=6) as work,
            tc.tile_pool(name="small", bufs=2) as small,
            tc.tile_pool(name="psum", bufs=2, space="PSUM") as psum,
            tc.tile_pool(name="psacc", bufs=4, space="PSUM") as psacc,
            tc.tile_pool(name="dram", bufs=1, space="DRAM") as dram,
        ):
            # ---- preloads ----
            idx1_sb = cpool.tile([P, tc1 * 8], mybir.dt.int16)
            nc.sync.dma_start(idx1_sb[:], idx1_in[:])
            ids1_sb = cpool.tile([P, tc1], mybir.dt.bfloat16)
            nc.sync.dma_start(ids1_sb[:], ids1_in[:])
            idx2_sb = cpool.tile([P, tc2 * 8], mybir.dt.int16)
            nc.sync.dma_start(idx2_sb[:], idx2_in[:])
            ids2_sb = cpool.tile([P, tc2], mybir.dt.bfloat16)
            nc.sync.dma_start(ids2_sb[:], ids2_in[:])
            colidx_sb = cpool.tile([P, P], mybir.dt.bfloat16)
            nc.sync.dma_start(colidx_sb[:], colidx_in[:])
            wlin_f32 = cpool.tile([P, OUT_CH], mybir.dt.float32)
            nc.sync.dma_start(wlin_f32[:], wlin[:])
            wlin_sb = cpool.tile([P, OUT_CH], mybir.dt.bfloat16)
            nc.vector.tensor_copy(wlin_sb[:], wlin_f32[:])
            degv_sb = cpool.tile([P, n_node_tiles], mybir.dt.float32)
            nc.sync.dma_start(degv_sb[:], degv_r[:])
            scale_e = cpool.tile([P, n_seg_tiles], mybir.dt.float32)
            nc.sync.dma_start(scale_e[:], dege_r[:])
            ident = cpool.tile([P, P], mybir.dt.float32)
            make_identity(nc, ident[:])
            colidx3 = colidx_sb[:].rearrange("p (o e) -> p o e", o=1)

            xp_local = dram.tile([ns_pad, OUT_CH], mybir.dt.bfloat16)
            xe_part = [
                dram.tile([q_tiles[q] * P, OUT_CH], mybir.dt.bfloat16,
                          name=f"xe_part{q}")
                for q in range(n_ar)
            ]
            xe_full = dram.tile([seg_pad, OUT_CH], mybir.dt.bfloat16)
            xe_red = [
                dram.tile([q_tiles[q] * P, OUT_CH], mybir.dt.bfloat16,
                          name=f"xe_red{q}", addr_space="Shared")
                for q in range(n_ar)
            ]

            # ---- projection: xp_local = bf16(x_shard @ wlin) ----
            for t in range(n_tiles_proj):
                rows = slice(t * P, (t + 1) * P)
                xt = small.tile([P, IN_CH], mybir.dt.float32, tag="xt")
                nc.sync.dma_start(xt[:], x_shard[rows, :])
                tp = psum.tile([P, P], mybir.dt.float32, space="PSUM",
                               tag="tp")
                nc.tensor.transpose(tp[:], xt[:], ident[:])
                xts = small.tile([P, P], mybir.dt.bfloat16, tag="xts")
                nc.vector.tensor_copy(xts[:], tp[:])
                xpp = psum.tile([P, OUT_CH], mybir.dt.float32, space="PSUM",
                                tag="xpp")
                nc.tensor.matmul(xpp[:], xts[:], wlin_sb[:], start=True,
                                 stop=True)
                xps = small.tile([P, OUT_CH], mybir.dt.bfloat16, tag="xps")
                nc.vector.tensor_copy(xps[:], xpp[:])
                nc.sync.dma_start(xp_local[rows, :], xps[:])

            # ---- generic segment-sum stage ----
            def seg_stage(t, ch, co_t, idx_sb, ids_sb, src_ap, scale_ap,
                          out_ap, out_dtype):
                n = ch * P
                g = work.tile([P, n], mybir.dt.bfloat16, tag="g")
                # split gathers into balanced calls of <=4 chunks (512
                # descriptors), round-robined over the 4 SWDGE queues so
                # descriptor generation runs on all gpsimd core-pairs
                # concurrently
                n_calls = _cdiv(ch, 4)
                base_w, rem_w = divmod(ch, n_calls)
                c0 = 0
                for ci in range(n_calls):
                    cw = base_w + (1 if ci < rem_w else 0)
                    gs = g[:, c0 * P:(c0 + cw) * P].rearrange(
                        "p (c e) -> p c e", e=P)
                    nc.gpsimd.dma_gather(
                        gs, src_ap,
                        idx_sb[:, (co_t + c0) * 8:(co_t + c0 + cw) * 8],
                        cw * P, cw * P, P,
                        queue_num=qctr[0] % N_QUEUES)
                    qctr[0] += 1
                    c0 += cw
                s = work.tile([P, n], mybir.dt.bfloat16, tag="s")
                s3 = s[:].rearrange("p (c e) -> p c e", e=P)
                nc.vector.tensor_tensor(
                    out=s3,
                    in0=ids_sb[:, co_t:co_t + ch].to_broadcast([P, ch, P]),
                    in1=colidx3.to_broadcast([P, ch, P]),
                    op=mybir.AluOpType.is_equal,
                )
                acc = psacc.tile([P, OUT_CH], mybir.dt.float32,
                                 space="PSUM", tag="acc")
                for c in range(ch):
                    nc.tensor.matmul(
                        acc[:], s[:, c * P:(c + 1) * P],
                        g[:, c * P:(c + 1) * P],
                        start=(c == 0), stop=(c == ch - 1))
                ev = work.tile([P, OUT_CH], out_dtype, tag="ev")
                nc.vector.tensor_scalar_mul(ev[:], acc[:], scale_ap)
                nc.sync.dma_start(out_ap, ev[:])

            # ---- stage 1 (+ chunked AllReduce) ----
            for q in range(n_ar):
                for t in range(q_tile_lo[q], q_tile_lo[q + 1]):
                    trel = t - q_tile_lo[q]
                    seg_stage(
                        t, int(chunks1[t]), int(co1[t]), idx1_sb, ids1_sb,
                        xp_local[:], scale_e[:, t:t + 1],
                        xe_part[q][trel * P:(trel + 1) * P, :],
                        mybir.dt.bfloat16)
                if USE_COLLECTIVE:
                    nc.gpsimd.collective_compute(
                        "AllReduce", mybir.AluOpType.add,
                        replica_groups=[list(range(n_cores))],
                        ins=[xe_part[q].opt()],
                        outs=[xe_red[q].opt()],
                    )
                    nc.sync.dma_start(
                        xe_full[q_tile_lo[q] * P:q_tile_lo[q + 1] * P, :],
                        xe_red[q][:])
                else:
                    nc.sync.dma_start(
                        xe_full[q_tile_lo[q] * P:q_tile_lo[q + 1] * P, :],
                        xe_part[q][:])

            # ---- stage 2 ----
            for t in range(n_node_tiles):
                seg_stage(
                    t, int(chunks2[t]), int(co2[t]), idx2_sb, ids2_sb,
                    xe_full[:], degv_sb[:, t:t + 1],
                    out_shard[t * P:(t + 1) * P, :], mybir.dt.float32)

    nc.compile()
    return nc


def _host_prep(X, Wlin, degE, degV, W, g1_src, g1_dst, n_cores=N_CORES):
    ns = N_NODES // n_cores
    ns_pad = _cdiv(ns, P) * P
    n_seg_tiles = _cdiv(N_HEDGES, P)
    seg_pad = n_seg_tiles * P
    n_node_tiles = ns_pad // P

    core_of = g1_src // ns

    # stage 1: per core, sorted by dst
    o1 = np.lexsort((g1_dst, core_of))
    src1, dst1, c1 = g1_src[o1], g1_dst[o1], core_of[o1]
    cb1 = np.searchsorted(c1, np.arange(n_cores + 1))
    tile_key1, gidx1, lid1 = [], [], []
    for c in range(n_cores):
        lo, hi = cb1[c], cb1[c + 1]
        d = dst1[lo:hi]
        tile_key1.append(d // P)
        gidx1.append(src1[lo:hi] - c * ns)
        lid1.append((d % P).astype(np.float32))
    chunks1, idx1_w, ids1_w = _prep_stage(
        tile_key1, gidx1, lid1, n_seg_tiles, n_cores)

    # stage 2: per core, sorted by src
    o2 = np.argsort(g1_src, kind="stable")
    src2, dst2 = g1_src[o2], g1_dst[o2]
    cb2 = np.searchsorted(src2, np.arange(n_cores + 1) * ns)
    tile_key2, gidx2, lid2 = [], [], []
    for c in range(n_cores):
        lo, hi = cb2[c], cb2[c + 1]
        s_local = src2[lo:hi] - c * ns
        tile_key2.append(s_local // P)
        gidx2.append(dst2[lo:hi])
        lid2.append((s_local % P).astype(np.float32))
    chunks2, idx2_w, ids2_w = _prep_stage(
        tile_key2, gidx2, lid2, n_node_tiles, n_cores)

    # rearranged scale vectors: column t holds values for tile t's rows;
    # degE is pre-multiplied by W (elementwise hyperedge weight)
    def col_tiles(v, pad_rows):
        vp = np.zeros(pad_rows, dtype=np.float32)
        vp[:v.shape[0]] = v.reshape(-1)
        return np.ascontiguousarray(vp.reshape(pad_rows // P, P).T)

    dege_r = col_tiles((degE * W).astype(np.float32), seg_pad)
    colidx = np.broadcast_to(
        np.arange(P, dtype=np.float32), (P, P)).astype(BF16)

    in_maps = []
    for c in range(n_cores):
        xs = np.zeros((ns_pad, IN_CH), dtype=np.float32)
        xs[:ns] = X[c * ns:(c + 1) * ns]
        in_maps.append({
            "x_shard": xs,
            "wlin": np.ascontiguousarray(Wlin, dtype=np.float32),
            "dege_r": dege_r,
            "degv_r": col_tiles(degV[c * ns:(c + 1) * ns], ns_pad),
            "colidx": np.ascontiguousarray(colidx),
            "idx1": idx1_w[c],
            "ids1": ids1_w[c],
            "idx2": idx2_w[c],
            "ids2": ids2_w[c],
        })
    return in_maps, chunks1, chunks2, ns, ns_pad, seg_pad


def run_impl(inputs: dict, trace: bool = False):
    X = np.asarray(inputs["X"], dtype=np.float32)
    Wlin = np.asarray(inputs["Wlin"], dtype=np.float32)
    degE = np.asarray(inputs["degE"], dtype=np.float32)
    degV = np.asarray(inputs["degV"], dtype=np.float32)
    W = np.asarray(inputs["W"], dtype=np.float32)
    g1_src = np.asarray(inputs["g1_src"], dtype=np.int64)
    g1_dst = np.asarray(inputs["g1_dst"], dtype=np.int64)

    in_maps, chunks1, chunks2, ns, ns_pad, seg_pad = _host_prep(
        X, Wlin, degE, degV, W, g1_src, g1_dst)
    nc = _build_program(ns_pad, seg_pad, chunks1, chunks2, N_CORES)
    res = run_bass_kernel_spmd(nc, in_maps, core_ids=list(range(N_CORES)),
                               trace=trace)
    out = np.concatenate(
        [res.results[c]["out_shard"][:ns] for c in range(N_CORES)], axis=0)
    return out, res


def kernel(**inputs) -> np.ndarray:
    out, _ = run_impl(inputs, trace=False)
    return out


# revision 10
# speedup vs baseline: 1.0994x; 1.0210x over previous
"""Trainium2 Bass kernel for DGL HGNNConv-style hypergraph message passing.

Computation (see problem reference):
    Xp = X @ Wlin                                   # [N, 128] @ [128, 128]
    Xe = segment_sum(Xp[g1_src], g1_dst, 25000)     # node -> hyperedge
    Xe = Xe * degE * W
    Xv = segment_sum(Xe[g1_dst], g1_src, 100000)    # hyperedge -> node
    Xv = Xv * degV

Distribution strategy (8 NeuronCores, node-range sharding):
  - Core m owns node rows [m*12500, (m+1)*12500) and all nnz entries whose
    src falls in that range (both stages use the same entry sharding).
  - Projection: each core computes Xp (bf16) for its own node shard only.
  - Stage 1: per-core entries sorted by dst; rows of the local Xp gathered
    per entry (dma_gather), segment-summed into a full-range partial Xe
    via data-dependent one-hot matmuls (PSUM accumulation), scaled by
    degE*W, then AllReduced across cores (4 chunks, overlapped).
  - Stage 2: per-core entries sorted by src; rows of the reduced Xe
    gathered per entry, segment-summed into the core's node tile,
    scaled by degV, written to the core's output shard.

Performance notes:
  - dma_gather descriptor generation runs on one gpsimd (Q7) core-pair
    selected by queue_num; round-robining calls across the 4 SWDGE queues
    runs four generations concurrently (~3.6x gather throughput).
  - Gather tables (Xp, reduced Xe) are stored bf16: halves gather HBM
    traffic and feeds the one-hot matmuls bf16 inputs directly (1-pass PE
    matmuls instead of 4-pass fp32).
  - The AllReduce writes straight into the shared full-Xe tensor slice
    per chunk (no DRAM->DRAM copy).

Segment-sum-as-matmul: for each chunk of 128 gathered rows G [128e x 128f]
and one-hot S [128e x 128s] (S[k, m] = 1 iff entry k belongs to local
segment m, built on-chip with is_equal against an iota tile), the matmul
S^T @ G accumulates the chunk into the 128-segment PSUM tile.
"""

import ml_dtypes
import numpy as np

import concourse.bass as bass
import concourse.bacc as bacc
import concourse.tile as tile
import concourse.mybir as mybir
from concourse.bass_utils import run_bass_kernel_spmd
from concourse.masks import make_identity

P = 128
N_CORES = 8
N_QUEUES = 4

N_NODES = 100000
N_HEDGES = 25000
IN_CH = 128
OUT_CH = 128
N_AR_CHUNKS = 4  # AllReduce split for overlap with stage-1 compute
USE_COLLECTIVE = True

BF16 = ml_dtypes.bfloat16


def _cdiv(a, b):
    return (a + b - 1) // b


def _wrap_idx16(idx_flat: np.ndarray) -> np.ndarray:
    """Pack a flat index array into the [128, n/16] int16 SBUF layout used
    by dma_gather: flat index i -> partition i%16, column i//16, replicated
    across the eight 16-partition stripes."""
    n = idx_flat.shape[0]
    assert n % 16 == 0
    blk = idx_flat.astype(np.int16).reshape(n // 16, 16).T  # [16, cols]
    return np.tile(blk, (8, 1))  # [128, cols]


def _prep_stage(tile_key, gather_idx, local_id, n_tiles, n_cores):
    """Build per-core padded gather-index / segment-id arrays with a chunk
    schedule that is uniform across cores (SPMD requires one program).

    tile_key: per-core arrays with the tile id per entry (nondecreasing).
    Returns (chunks [n_tiles], idx_wrapped list, ids list).
    """
    counts = np.zeros((n_cores, n_tiles), dtype=np.int64)
    slices = []
    for c in range(n_cores):
        bounds = np.searchsorted(tile_key[c], np.arange(n_tiles + 1),
                                 side="left")
        counts[c] = bounds[1:] - bounds[:-1]
        slices.append(bounds)
    chunks = np.maximum(1, _cdiv(counts.max(axis=0), P)).astype(np.int64)
    total_chunks = int(chunks.sum())
    total = total_chunks * P
    co = np.concatenate([[0], np.cumsum(chunks)])

    idx_w, ids_w = [], []
    for c in range(n_cores):
        idx_flat = np.zeros(total, dtype=np.int16)
        ids_flat = np.full(total, -1.0, dtype=np.float32)
        bounds = slices[c]
        gi, li = gather_idx[c], local_id[c]
        for t in range(n_tiles):
            lo, hi = bounds[t], bounds[t + 1]
            cnt = hi - lo
            base = int(co[t]) * P
            idx_flat[base:base + cnt] = gi[lo:hi]
            ids_flat[base:base + cnt] = li[lo:hi]
        idx_w.append(_wrap_idx16(idx_flat))
        ids_w.append(np.ascontiguousarray(
            ids_flat.reshape(total_chunks, P).T).astype(BF16))
    return chunks, idx_w, ids_w


def _build_program(ns_pad, seg_pad, chunks1, chunks2, n_cores):
    """Emit the SPMD Bass program (identical for all cores)."""
    n_tiles_proj = ns_pad // P
    n_seg_tiles = seg_pad // P
    n_node_tiles = ns_pad // P
    tc1 = int(chunks1.sum())
    tc2 = int(chunks2.sum())
    co1 = np.concatenate([[0], np.cumsum(chunks1)]).astype(int)
    co2 = np.concatenate([[0], np.cumsum(chunks2)]).astype(int)

    nc = bacc.Bacc("TRN2", target_bir_lowering=False, debug=False,
                   num_devices=n_cores, num_swdge_queues=N_QUEUES)

    x_shard = nc.dram_tensor("x_shard", [ns_pad, IN_CH], mybir.dt.float32,
                             kind="ExternalInput")
    wlin = nc.dram_tensor("wlin", [IN_CH, OUT_CH], mybir.dt.float32,
                          kind="ExternalInput")
    dege_r = nc.dram_tensor("dege_r", [P, n_seg_tiles], mybir.dt.float32,
                            kind="ExternalInput")
    degv_r = nc.dram_tensor("degv_r", [P, n_node_tiles], mybir.dt.float32,
                            kind="ExternalInput")
    colidx_in = nc.dram_tensor("colidx", [P, P], mybir.dt.bfloat16,
                               kind="ExternalInput")
    idx1_in = nc.dram_tensor("idx1", [P, tc1 * 8], mybir.dt.int16,
                             kind="ExternalInput")
    ids1_in = nc.dram_tensor("ids1", [P, tc1], mybir.dt.bfloat16,
                             kind="ExternalInput")
    idx2_in = nc.dram_tensor("idx2", [P, tc2 * 8], mybir.dt.int16,
                             kind="ExternalInput")
    ids2_in = nc.dram_tensor("ids2", [P, tc2], mybir.dt.bfloat16,
                             kind="ExternalInput")
    out_shard = nc.dram_tensor("out_shard", [ns_pad, OUT_CH],
                               mybir.dt.float32, kind="ExternalOutput")

    # AllReduce chunk row ranges (in seg tiles): front-loaded groups with a
    # small final group so the pipeline-drain tail before the last AllReduce
    # is short
    n_ar = min(N_AR_CHUNKS, n_seg_tiles)
    if n_ar == 4 and n_seg_tiles == 196:
        q_tiles = [56, 56, 56, 28]
    else:
        q_tiles = [n_seg_tiles // n_ar] * n_ar
        for i in range(n_seg_tiles % n_ar):
            q_tiles[i] += 1
    q_tile_lo = np.concatenate([[0], np.cumsum(q_tiles)]).astype(int)

    qctr = [0]  # SWDGE queue rotation

    with tile.TileContext(nc) as tc:
        with (
            tc.tile_pool(name="const", bufs=1) as cpool,
            tc.tile_pool(name="work", bufs=8) as work,
            tc.tile_pool(name="small", bufs=2) as small,
            tc.tile_pool(name="psum", bufs=2, space="PSUM") as psum,
            tc.tile_pool(name="psacc", bufs=6, space="PSUM") as psacc,
            tc.tile_pool(name="dram", bufs=1, space="DRAM") as dram,
        ):
            # ---- preloads ----
            idx1_sb = cpool.tile([P, tc1 * 8], mybir.dt.int16)
            nc.sync.dma_start(idx1_sb[:], idx1_in[:])
            ids1_sb = cpool.tile([P, tc1], mybir.dt.bfloat16)
            nc.sync.dma_start(ids1_sb[:], ids1_in[:])
            idx2_sb = cpool.tile([P, tc2 * 8], mybir.dt.int16)
            nc.sync.dma_start(idx2_sb[:], idx2_in[:])
            ids2_sb = cpool.tile([P, tc2], mybir.dt.bfloat16)
            nc.sync.dma_start(ids2_sb[:], ids2_in[:])
            colidx_sb = cpool.tile([P, P], mybir.dt.bfloat16)
            nc.sync.dma_start(colidx_sb[:], colidx_in[:])
            wlin_f32 = cpool.tile([P, OUT_CH], mybir.dt.float32)
            nc.sync.dma_start(wlin_f32[:], wlin[:])
            wlin_sb = cpool.tile([P, OUT_CH], mybir.dt.bfloat16)
            nc.vector.tensor_copy(wlin_sb[:], wlin_f32[:])
            degv_sb = cpool.tile([P, n_node_tiles], mybir.dt.float32)
            nc.sync.dma_start(degv_sb[:], degv_r[:])
            scale_e = cpool.tile([P, n_seg_tiles], mybir.dt.float32)
            nc.sync.dma_start(scale_e[:], dege_r[:])
            ident = cpool.tile([P, P], mybir.dt.float32)
            make_identity(nc, ident[:])
            colidx3 = colidx_sb[:].rearrange("p (o e) -> p o e", o=1)

            xp_local = dram.tile([ns_pad, OUT_CH], mybir.dt.bfloat16)
            xe_part = [
                dram.tile([q_tiles[q] * P, OUT_CH], mybir.dt.bfloat16,
                          name=f"xe_part{q}")
                for q in range(n_ar)
            ]
            xe_full = dram.tile([seg_pad, OUT_CH], mybir.dt.bfloat16)
            xe_red = [
                dram.tile([q_tiles[q] * P, OUT_CH], mybir.dt.bfloat16,
                          name=f"xe_red{q}", addr_space="Shared")
                for q in range(n_ar)
            ]

            # ---- projection: xp_local = bf16(x_shard @ wlin) ----
            for t in range(n_tiles_proj):
                rows = slice(t * P, (t + 1) * P)
                xt = small.tile([P, IN_CH], mybir.dt.float32, tag="xt")
                nc.sync.dma_start(xt[:], x_shard[rows, :])
                tp = psum.tile([P, P], mybir.dt.float32, space="PSUM",
                               tag="tp")
                nc.tensor.transpose(tp[:], xt[:], ident[:])
                xts = small.tile([P, P], mybir.dt.bfloat16, tag="xts")
                nc.vector.tensor_copy(xts[:], tp[:])
                xpp = psum.tile([P, OUT_CH], mybir.dt.float32, space="PSUM",
                                tag="tp")
                nc.tensor.matmul(xpp[:], xts[:], wlin_sb[:], start=True,
                                 stop=True)
                xps = small.tile([P, OUT_CH], mybir.dt.bfloat16, tag="xps")
                nc.vector.tensor_copy(xps[:], xpp[:])
                nc.sync.dma_start(xp_local[rows, :], xps[:])

            # ---- generic segment-sum stage ----
            def seg_stage(t, ch, co_t, idx_sb, ids_sb, src_ap, scale_ap,
                          out_ap, out_dtype):
                n = ch * P
                g = work.tile([P, n], mybir.dt.bfloat16, tag="g")
                # split gathers into balanced calls of <=4 chunks (512
                # descriptors), round-robined over the 4 SWDGE queues so
                # descriptor generation runs on all gpsimd core-pairs
                # concurrently
                n_calls = _cdiv(ch, 4)
                base_w, rem_w = divmod(ch, n_calls)
                c0 = 0
                for ci in range(n_calls):
                    cw = base_w + (1 if ci < rem_w else 0)
                    gs = g[:, c0 * P:(c0 + cw) * P].rearrange(
                        "p (c e) -> p c e", e=P)
                    nc.gpsimd.dma_gather(
                        gs, src_ap,
                        idx_sb[:, (co_t + c0) * 8:(co_t + c0 + cw) * 8],
                        cw * P, cw * P, P,
                        queue_num=qctr[0] % N_QUEUES)
                    qctr[0] += 1
                    c0 += cw
                s = work.tile([P, n], mybir.dt.bfloat16, tag="s")
                s3 = s[:].rearrange("p (c e) -> p c e", e=P)
                nc.vector.tensor_tensor(
                    out=s3,
                    in0=ids_sb[:, co_t:co_t + ch].to_broadcast([P, ch, P]),
                    in1=colidx3.to_broadcast([P, ch, P]),
                    op=mybir.AluOpType.is_equal,
                )
                acc = psacc.tile([P, OUT_CH], mybir.dt.float32,
                                 space="PSUM", tag="acc")
                for c in range(ch):
                    nc.tensor.matmul(
                        acc[:], s[:, c * P:(c + 1) * P],
                        g[:, c * P:(c + 1) * P],
                        start=(c == 0), stop=(c == ch - 1))
                ev = work.tile([P, OUT_CH], out_dtype, tag="ev")
                nc.vector.tensor_scalar_mul(ev[:], acc[:], scale_ap)
                nc.sync.dma_start(out_ap, ev[:])

            # ---- stage 1 (+ chunked AllReduce) ----
            # Each group's collective is issued DELAY_TILES into the next
            # group: by then the group's evac DMAs have drained, so the
            # collective's input wait doesn't head-block the gpsimd queue
            # (which would stall gather dispatch).
            DELAY_TILES = 8

            def emit_ar(q):
                if USE_COLLECTIVE:
                    nc.gpsimd.collective_compute(
                        "AllReduce", mybir.AluOpType.add,
                        replica_groups=[list(range(n_cores))],
                        ins=[xe_part[q].opt()],
                        outs=[xe_red[q].opt()],
                    )
                    nc.sync.dma_start(
                        xe_full[q_tile_lo[q] * P:q_tile_lo[q + 1] * P, :],
                        xe_red[q][:])
                else:
                    nc.sync.dma_start(
                        xe_full[q_tile_lo[q] * P:q_tile_lo[q + 1] * P, :],
                        xe_part[q][:])

            for q in range(n_ar):
                for t in range(q_tile_lo[q], q_tile_lo[q + 1]):
                    trel = t - q_tile_lo[q]
                    if q > 0 and t == q_tile_lo[q] + DELAY_TILES:
                        emit_ar(q - 1)
                    seg_stage(
                        t, int(chunks1[t]), int(co1[t]), idx1_sb, ids1_sb,
                        xp_local[:], scale_e[:, t:t + 1],
                        xe_part[q][trel * P:(trel + 1) * P, :],
                        mybir.dt.bfloat16)
                if q == n_ar - 1:
                    emit_ar(q)

            # ---- stage 2 ----
            for t in range(n_node_tiles):
                seg_stage(
                    t, int(chunks2[t]), int(co2[t]), idx2_sb, ids2_sb,
                    xe_full[:], degv_sb[:, t:t + 1],
                    out_shard[t * P:(t + 1) * P, :], mybir.dt.float32)

    nc.compile()
    return nc


def _host_prep(X, Wlin, degE, degV, W, g1_src, g1_dst, n_cores=N_CORES):
    ns = N_NODES // n_cores
    ns_pad = _cdiv(ns, P) * P
    n_seg_tiles = _cdiv(N_HEDGES, P)
    seg_pad = n_seg_tiles * P
    n_node_tiles = ns_pad // P

    core_of = g1_src // ns

    # stage 1: per core, sorted by dst
    o1 = np.lexsort((g1_dst, core_of))
    src1, dst1, c1 = g1_src[o1], g1_dst[o1], core_of[o1]
    cb1 = np.searchsorted(c1, np.arange(n_cores + 1))
    tile_key1, gidx1, lid1 = [], [], []
    for c in range(n_cores):
        lo, hi = cb1[c], cb1[c + 1]
        d = dst1[lo:hi]
        tile_key1.append(d // P)
        gidx1.append(src1[lo:hi] - c * ns)
        lid1.append((d % P).astype(np.float32))
    chunks1, idx1_w, ids1_w = _prep_stage(
        tile_key1, gidx1, lid1, n_seg_tiles, n_cores)

    # stage 2: per core, sorted by src
    o2 = np.argsort(g1_src, kind="stable")
    src2, dst2 = g1_src[o2], g1_dst[o2]
    cb2 = np.searchsorted(src2, np.arange(n_cores + 1) * ns)
    tile_key2, gidx2, lid2 = [], [], []
    for c in range(n_cores):
        lo, hi = cb2[c], cb2[c + 1]
        s_local = src2[lo:hi] - c * ns
        tile_key2.append(s_local // P)
        gidx2.append(dst2[lo:hi])
        lid2.append((s_local % P).astype(np.float32))
    chunks2, idx2_w, ids2_w = _prep_stage(
        tile_key2, gidx2, lid2, n_node_tiles, n_cores)

    # rearranged scale vectors: column t holds values for tile t's rows;
    # degE is pre-multiplied by W (elementwise hyperedge weight)
    def col_tiles(v, pad_rows):
        vp = np.zeros(pad_rows, dtype=np.float32)
        vp[:v.shape[0]] = v.reshape(-1)
        return np.ascontiguousarray(vp.reshape(pad_rows // P, P).T)

    dege_r = col_tiles((degE * W).astype(np.float32), seg_pad)
    colidx = np.broadcast_to(
        np.arange(P, dtype=np.float32), (P, P)).astype(BF16)

    in_maps = []
    for c in range(n_cores):
        xs = np.zeros((ns_pad, IN_CH), dtype=np.float32)
        xs[:ns] = X[c * ns:(c + 1) * ns]
        in_maps.append({
            "x_shard": xs,
            "wlin": np.ascontiguousarray(Wlin, dtype=np.float32),
            "dege_r": dege_r,
            "degv_r": col_tiles(degV[c * ns:(c + 1) * ns], ns_pad),
            "colidx": np.ascontiguousarray(colidx),
            "idx1": idx1_w[c],
            "ids1": ids1_w[c],
            "idx2": idx2_w[c],
            "ids2": ids2_w[c],
        })
    return in_maps, chunks1, chunks2, ns, ns_pad, seg_pad


def run_impl(inputs: dict, trace: bool = False):
    X = np.asarray(inputs["X"], dtype=np.float32)
    Wlin = np.asarray(inputs["Wlin"], dtype=np.float32)
    degE = np.asarray(inputs["degE"], dtype=np.float32)
    degV = np.asarray(inputs["degV"], dtype=np.float32)
    W = np.asarray(inputs["W"], dtype=np.float32)
    g1_src = np.asarray(inputs["g1_src"], dtype=np.int64)
    g1_dst = np.asarray(inputs["g1_dst"], dtype=np.int64)

    in_maps, chunks1, chunks2, ns, ns_pad, seg_pad = _host_prep(
        X, Wlin, degE, degV, W, g1_src, g1_dst)
    nc = _build_program(ns_pad, seg_pad, chunks1, chunks2, N_CORES)
    res = run_bass_kernel_spmd(nc, in_maps, core_ids=list(range(N_CORES)),
                               trace=trace)
    out = np.concatenate(
        [res.results[c]["out_shard"][:ns] for c in range(N_CORES)], axis=0)
    return out, res


def kernel(**inputs) -> np.ndarray:
    out, _ = run_impl(inputs, trace=False)
    return out
